# revision 51
# baseline (speedup 1.0000x reference)
"""Trainium2 Bass kernel for multi-head attention (b=4, n=2048, d=512, h=8, dk=dv=64).

Sharding: 8 cores = 4 batches x 2 query-halves. Each core computes K/V for its
full batch sequence (2048) and attention outputs for its 1024 query rows.
No collectives needed; host stacks the per-core [1024, 512] outputs.

Per-core dataflow:
  x^T [512, 2048] staged in SBUF as bf16; projections (bf16 MMs, f32 PSUM)
  emission-ordered against the HBM input stream.  Q/K projections are
  head-PAIR packed: one [128 = h_even dims | h_odd dims] PSUM tile per pair
  covers two heads per moving stream (halved MM columns, unreplicated wq/wk).
  Per-head S^T keeps full 128x128 stationaries via the zero-half trick:
  qt_h = [q+bias; 0] (or flipped) against a pair-shared kt, so the dead qt
  half contracts the other head's K rows away.  S^T/PV in f32r/bf16; exp on
  ScalarE from PSUM per [128,1024] chunk (the phase pacer).  The whole
  attention runs as ONE flat 128-chunk software pipeline: PV trails S^T by
  LOOK chunks across head boundaries, softmax-normalize runs inline, and all
  remaining projection + output-projection work is drip-fed between chunks in
  <=2-matmul slices from a dedicated PSUM pool so the exp stream never
  stalls.  Output projection accumulates per head-pair into SBUF (y_acc);
  the tail only adds pair 3 (h6's half early, h7's after its normalize) and
  streams y out in fp16.
"""
import numpy as np

B, N, MODEL = 4, 2048, 512
H, DK = 8, 64
SCALE = DK ** -0.5
NI = 1024           # query rows per core
NCH = MODEL // 128  # model-dim chunks
NJC = N // 128      # key/value chunks
NHP = H // 2        # head pairs
LOOK = 9            # PV chunk lookahead behind S^T

_COMPILED = None


def _build():
    import concourse.bass as bass
    from concourse import bacc
    import concourse.mybir as mybir
    import concourse.tile as tile

    F32 = mybir.dt.float32
    F32R = mybir.dt.float32r
    BF16 = mybir.dt.bfloat16
    F16 = mybir.dt.float16
    EXP = mybir.ActivationFunctionType.Exp
    AID = mybir.ActivationFunctionType.Identity

    nc = bacc.Bacc("TRN2", target_bir_lowering=False, debug=False, num_devices=8)
    xt_in = nc.dram_tensor("xt", [MODEL, N], BF16, kind="ExternalInput")
    wq_in = nc.dram_tensor("wq", [MODEL, MODEL], BF16, kind="ExternalInput")
    wk_in = nc.dram_tensor("wk", [MODEL, MODEL], BF16, kind="ExternalInput")
    wv_in = nc.dram_tensor("wv", [MODEL, MODEL], BF16, kind="ExternalInput")
    relb_in = nc.dram_tensor("relb", [128, NHP], F32, kind="ExternalInput")
    wo_in = nc.dram_tensor("wo", [MODEL, MODEL], F32R, kind="ExternalInput")
    bo_in = nc.dram_tensor("bo", [1, MODEL], F32, kind="ExternalInput")
    onesb_in = nc.dram_tensor("onesb", [128, NJC * H], BF16, kind="ExternalInput")
    y_out = nc.dram_tensor("y", [NI, MODEL], F16, kind="ExternalOutput")

    with tile.TileContext(nc) as tc:
        with (
            tc.tile_pool(name="w", bufs=1) as wp,
            tc.tile_pool(name="acts", bufs=1) as ap,
            tc.tile_pool(name="big", bufs=2, space="PSUM") as ps,
            tc.tile_pool(name="qk", bufs=2, space="PSUM") as qkp,
        ):
            # ---------- persistent tiles ----------
            wo = wp.tile([128, NCH, MODEL], F32R, tag="wo")
            bo = wp.tile([1, MODEL], F32, tag="bo")
            bo_b = wp.tile([128, MODEL], F32, tag="bo_b")
            vv_a = ap.tile([128, NJC // 2, H * 65], BF16, tag="vva")
            vv_b = ap.tile([128, NJC // 2, H * 65], BF16, tag="vvb")
            def vvt(jc):
                return (vv_a if jc < NJC // 2 else vv_b)[:, jc % (NJC // 2)]
            relb = ap.tile([128, NHP], F32, tag="relb")
            outt = ap.tile([128, NCH, NI], F32R, tag="outt")
            kt = ap.tile([128, NHP, NJC, 128], F32R, tag="kt")
            qt = ap.tile([128, H, NI], F32R, tag="qt")

            def r3(d):
                return d[:].rearrange("(c p) n -> p c n", p=128)

            dma_n = [0]
            def dma(out, in_):
                engs = (nc.sync, nc.gpsimd, nc.scalar)
                engs[dma_n[0] % 3].dma_start(out=out, in_=in_)
                dma_n[0] += 1
            def dma2(out, in_):
                # split in half over two queues for faster arrival
                dma(out[:, 0:2], in_[:, 0:2])
                dma(out[:, 2:4], in_[:, 2:4])

            with tc.tile_pool(name="proj", bufs=1) as pp:
                xt0 = pp.tile([128, NCH, 512], BF16, tag="xt0")
                xt1 = pp.tile([128, NCH, 512], BF16, tag="xt1")
                xt2 = pp.tile([128, NCH, 512], BF16, tag="xt2")
                xt3 = pp.tile([128, NCH, 512], BF16, tag="xt3")
                xts = [xt0, xt1, xt2, xt3]
                wq = pp.tile([128, NCH, MODEL], BF16, tag="wq")
                wk = pp.tile([128, NCH, MODEL], BF16, tag="wk")
                wv = pp.tile([128, NCH, MODEL], BF16, tag="wv")
                onesb_t = pp.tile([128, NJC * H], BF16, tag="onesb")

                # ---- DMA emission: one descriptor per tensor, priority
                # order round-robined over 3 queues ----
                xsrc = r3(xt_in)
                def dma_x(q):
                    dma2(xts[q][:], xsrc[:, :, q * 512:(q + 1) * 512])
                dma(relb[:], relb_in[:])
                dma(bo[:], bo_in[:])
                dma(onesb_t[:], onesb_in[:])
                dma2(wv[:], r3(wv_in))
                dma_x(0)
                dma(wq[:, :, 0:128], r3(wq_in)[:, :, 0:128])
                dma_x(1)
                dma(wk[:, :, 0:128], r3(wk_in)[:, :, 0:128])
                dma_x(2)
                dma_x(3)
                dma2(wq[:, :, 128:512], r3(wq_in)[:, :, 128:512])
                dma2(wk[:, :, 128:512], r3(wk_in)[:, :, 128:512])
                dma(wo[:], r3(wo_in))
                # HAM warm-up: accumulating matmuls on a zeroed scratch keep
                # the PE activity monitor busy while the input stream lands
                warm = pp.tile([128, 640], BF16, tag="warm")
                nc.vector.memset(warm[:], 0.0)
                w_ps = qkp.tile([128, MODEL], F32, tag="qk", name="w_ps")
                for w in range(12):
                    nc.tensor.matmul(w_ps[:], warm[:, 0:128], warm[:, 128:640],
                                     start=(w == 0), stop=(w == 11))
                # zero qt up front (the dead half of the zero-half trick must
                # be zero to mask the other head's K rows in the shared kt;
                # live halves are overwritten by the Q projection drains)
                for h in range(H):
                    nc.gpsimd.memset(qt[:, h, :].bitcast(F32), 0.0)
                nc.gpsimd.partition_broadcast(bo_b[:], bo[:])
                # ones columns of V_aug: contiguous DMA to scratch, strided copy
                for vh in range(2):
                    nc.vector.tensor_copy(
                        (vv_a if vh == 0 else vv_b)[:]
                        .rearrange("p j (h e) -> p (j h) e", e=65)[:, :, 64:65],
                        onesb_t[:, vh * NJC * H // 2:(vh + 1) * NJC * H // 2]
                        .rearrange("p (n o) -> p n o", o=1))

                def xtv(ch, start, size):
                    t = xts[start // 512]
                    off = start % 512
                    assert off + size <= 512
                    return t[:, ch, off:off + size]

                # ---- projection emitters, sliceable into 2-MM halves ----
                vps_st, qps_st, kps_st = {}, {}, {}

                def emit_v(jc, half):
                    if half == 0:
                        v_ps = qkp.tile([128, MODEL], F32, tag="qk",
                                        name="v_ps")
                        vps_st[jc] = v_ps
                    else:
                        v_ps = vps_st.pop(jc)
                    for ch in ((0, 1) if half == 0 else (2, 3)):
                        nc.tensor.matmul(v_ps[:],
                                         xtv(ch, jc * 128, 128),
                                         wv[:, ch],
                                         start=(ch == 0), stop=(ch == NCH - 1))
                    if half == 1:
                        nc.vector.tensor_copy(
                            vvt(jc).rearrange("p (h e) -> p h e", e=65)[:, :, 0:64],
                            v_ps[:].rearrange("p (h e) -> p h e", e=64))

                def emit_q(hp, ib, half, eng=None):
                    if half == 0:
                        q_ps = qkp.tile([128, MODEL], F32, tag="qk",
                                        name="q_ps")
                        qps_st[(hp, ib)] = q_ps
                    else:
                        q_ps = qps_st.pop((hp, ib))
                    for ch in ((0, 1) if half == 0 else (2, 3)):
                        nc.tensor.matmul(
                            q_ps[:, 0:512],
                            wq[:, ch, hp * 128:(hp + 1) * 128],
                            xtv(ch, ib * 512, 512),
                            start=(ch == 0), stop=(ch == NCH - 1))
                    if half == 1:
                        isl = slice(ib * 512, ib * 512 + 512)
                        if eng is None:
                            nc.vector.tensor_scalar_add(
                                qt[0:64, 2 * hp, isl], q_ps[0:64, 0:512],
                                relb[0:64, hp:hp + 1])
                            nc.vector.tensor_scalar_add(
                                qt[64:128, 2 * hp + 1, isl],
                                q_ps[64:128, 0:512],
                                relb[64:128, hp:hp + 1])
                        else:
                            eng.activation(qt[0:64, 2 * hp, isl],
                                           q_ps[0:64, 0:512], AID,
                                           bias=relb[0:64, hp:hp + 1])
                            eng.activation(qt[64:128, 2 * hp + 1, isl],
                                           q_ps[64:128, 0:512], AID,
                                           bias=relb[64:128, hp:hp + 1])

                def emit_k(hp, jb, sb, half, eng=None):
                    if half == 0:
                        k_ps = qkp.tile([128, MODEL], F32, tag="qk",
                                        name="k_ps")
                        kps_st[(hp, jb, sb)] = k_ps
                    else:
                        k_ps = kps_st.pop((hp, jb, sb))
                    off = jb * NI + sb * 512
                    for ch in ((0, 1) if half == 0 else (2, 3)):
                        nc.tensor.matmul(
                            k_ps[:, 0:512],
                            wk[:, ch, hp * 128:(hp + 1) * 128],
                            xtv(ch, off, 512),
                            start=(ch == 0), stop=(ch == NCH - 1))
                    if half == 1:
                        jcs = slice(jb * 8 + sb * 4, jb * 8 + sb * 4 + 4)
                        src = k_ps[:, 0:512].rearrange("p (j m) -> p j m", m=128)
                        if eng is None:
                            nc.vector.tensor_copy(kt[0:64, hp, jcs, :],
                                                  src[0:64])
                            nc.vector.tensor_copy(kt[64:128, hp, jcs, :],
                                                  src[64:128])
                        else:
                            eng.copy(kt[0:64, hp, jcs, :], src[0:64])
                            eng.copy(kt[64:128, hp, jcs, :], src[64:128])

                # ---- pre-ST: V0-5, K(0,jb0), Q0; K(0,jb1) goes to the
                # interleave (its x3 dependency lands after S^T starts) ----
                for jc in range(6):
                    emit_v(jc, 0)
                    emit_v(jc, 1)
                for sb in range(2):
                    emit_k(0, 0, sb, 0)
                    emit_k(0, 0, sb, 1, eng=nc.scalar)
                for ib in range(2):
                    emit_q(0, ib, 0)
                    emit_q(0, ib, 1, eng=nc.scalar)

                # ------ attention: flat pipeline over 128 S^T chunks with
                # drip-fed interleave ops ------
                with tc.tile_pool(name="pt", bufs=12) as ptp, \
                     tc.tile_pool(name="pv", bufs=1, space="PSUM") as pvp, \
                     tc.tile_pool(name="norm", bufs=2) as np_, \
                     tc.tile_pool(name="yac", bufs=1) as yac, \
                     tc.tile_pool(name="ysb", bufs=2) as yp_sb:
                    pts = {}
                    pvs = {}
                    y_acc = yac.tile([128, NI // 128, MODEL], F16, tag="yacc")

                    def emit_y(p, ib, hh):
                        # hh: None = full 128-row contraction (pairs 0-2);
                        # 0/1 = one head's 64 rows (pair 3 split around h7)
                        y_ps = qkp.tile([128, MODEL], F32, tag="qk",
                                        name="y_ps")
                        if hh is None:
                            nc.tensor.matmul(
                                y_ps[:], outt[:, p, ib * 128:(ib + 1) * 128],
                                wo[:, p], start=True, stop=True)
                        else:
                            b0 = hh * 64
                            nc.tensor.matmul(
                                y_ps[:],
                                outt[b0:b0 + 64, p, ib * 128:(ib + 1) * 128],
                                wo[b0:b0 + 64, p], start=True, stop=True)
                        if p == 0:
                            nc.vector.tensor_tensor(
                                out=y_acc[:, ib], in0=y_ps[:],
                                in1=bo_b[:], op=mybir.AluOpType.add)
                        elif p == 3:
                            y_sb = yp_sb.tile([128, MODEL], F16, tag="ysb")
                            nc.vector.tensor_tensor(
                                out=y_sb[:], in0=y_ps[:],
                                in1=y_acc[:, ib], op=mybir.AluOpType.add)
                            dma(y_out[ib * 128:(ib + 1) * 128, :], y_sb[:])
                        else:
                            nc.vector.tensor_tensor(
                                out=y_acc[:, ib], in0=y_ps[:],
                                in1=y_acc[:, ib], op=mybir.AluOpType.add)

                    def emit_st(g):
                        h, jc = g // NJC, g % NJC
                        st = ps.tile([128, NI], F32, tag="big")
                        for ih in range(2):
                            nc.tensor.matmul(
                                st[:, ih * 512:(ih + 1) * 512],
                                kt[:, h // 2, jc],
                                qt[:, h, ih * 512:(ih + 1) * 512],
                                start=True, stop=True)
                        pt = ptp.tile([128, NI], BF16, tag="pt")
                        pts[g] = pt
                        nc.scalar.activation(pt[:], st[:], EXP, scale=1.0)

                    def emit_pv(g):
                        h, jc = g // NJC, g % NJC
                        if jc == 0:
                            pv_t = pvp.tile([65, NI], F32, tag="pv",
                                            name="pv_t")
                            pvs[h] = pv_t
                        else:
                            pv_t = pvs[h]
                        pt = pts.pop(g)
                        for ih in range(2):
                            nc.tensor.matmul(
                                pv_t[:, ih * 512:(ih + 1) * 512],
                                vvt(jc)[:, h * 65:(h + 1) * 65],
                                pt[:, ih * 512:(ih + 1) * 512],
                                start=(jc == 0), stop=(jc == NJC - 1))
                        if jc == NJC - 1:
                            emit_norm(h)

                    def emit_norm(h):
                        hp, base = h // 2, (h % 2) * 64
                        pv_t = pvs.pop(h)
                        if h < 7:
                            # stage numerator to SBUF so the PSUM tile frees
                            # before the broadcast chain finishes
                            num = np_.tile([64, NI], F32, tag="num")
                            nc.vector.tensor_copy(num[:], pv_t[0:64, :])
                            num = num[:]
                        else:
                            num = pv_t[0:64, :]
                        den = np_.tile([1, NI], F32, tag="den")
                        nc.vector.tensor_copy(den[:], pv_t[64:65, :])
                        rrow = np_.tile([1, NI], F32, tag="rrow")
                        nc.vector.reciprocal_approx_fast(rrow[:], den[:])
                        rb = np_.tile([64, NI], F32, tag="rb")
                        nc.gpsimd.partition_broadcast(rb[:], rrow[:])
                        nc.vector.tensor_tensor(
                            out=outt[base:base + 64, hp, :],
                            in0=num, in1=rb[:],
                            op=mybir.AluOpType.mult)

                    # ---- interleave schedule: one <=2-MM op per slot,
                    # earliest-deadline-first ----
                    ilv = {}
                    def at(g, fn):
                        ilv.setdefault(g, []).append(fn)
                    for sb in range(2):           # K(0,jb1) quarters
                        for half in range(2):
                            at(1 + 2 * sb + half,
                               lambda sb=sb, half=half: emit_k(0, 1, sb, half))
                    for jc in range(6, 16):       # V chunks 6..15
                        at(2 * jc - 7, lambda jc=jc: emit_v(jc, 0))
                        at(2 * jc - 6, lambda jc=jc: emit_v(jc, 1))
                    def sched_qk(hp, g0):
                        g = g0
                        for ib in range(2):
                            for half in range(2):
                                at(g, lambda hp=hp, ib=ib, half=half:
                                   emit_q(hp, ib, half))
                                g += 1
                        for jb in range(2):
                            for sb in range(2):
                                for half in range(2):
                                    at(g, lambda hp=hp, jb=jb, sb=sb,
                                       half=half: emit_k(hp, jb, sb, half))
                                    g += 1
                    sched_qk(1, 25)
                    sched_qk(2, 56)
                    sched_qk(3, 88)
                    # y-partials trail pair p's normalize (g = 32p + 40)
                    gy = {0: range(41, 57, 2), 1: range(73, 89, 2),
                          2: range(105, 121, 2)}
                    for p, gs in gy.items():
                        for ib, g in enumerate(gs):
                            assert len(ilv.get(g, ())) == 0 or g >= 41
                            at(g, lambda p=p, ib=ib: emit_y(p, ib, None))
                    pv_next = [0]
                    def drain_pv(upto):
                        while pv_next[0] <= min(upto, H * NJC - 1):
                            emit_pv(pv_next[0])
                            pv_next[0] += 1
                    for g in range(H * NJC):
                        emit_st(g)
                        for fn in ilv.get(g, ()):
                            fn()
                        # PV trails by LOOK; near the end, catch up so the
                        # drain overlaps the last exps
                        lag = LOOK if g < 120 else LOOK - (g - 119)
                        drain_pv(g - max(lag, 1))
                    drain_pv(H * NJC - 1)
                    # keep the PE clock warm through h7's normalize so the
                    # final output-projection matmuls run at full rate
                    w_ps2 = qkp.tile([128, MODEL], F32, tag="qk", name="w_ps2")
                    for w in range(14):
                        nc.tensor.matmul(w_ps2[:], warm[:, 0:128],
                                         warm[:, 128:640],
                                         start=(w == 0), stop=(w == 13))

                    # ---- tail: pair 3 + writeback ----
                    for ib in range(8):
                        emit_y(3, ib, None)

    nc.compile()
    return nc


def _get_compiled():
    global _COMPILED
    if _COMPILED is None:
        _COMPILED = _build()
    return _COMPILED


def kernel(x, Wq, Wk, Wv, Wo, bo, rel_content_bias, _trace=False):
    from concourse.bass_utils import run_bass_kernel_spmd
    import ml_dtypes

    nc = _get_compiled()
    BF = ml_dtypes.bfloat16

    x = np.asarray(x, dtype=np.float32)
    Wq = np.asarray(Wq, dtype=np.float32)
    Wk = np.asarray(Wk, dtype=np.float32)
    Wv = np.asarray(Wv, dtype=np.float32)
    Wo = np.asarray(Wo, dtype=np.float32)
    bo = np.asarray(bo, dtype=np.float32)
    bias = np.asarray(rel_content_bias, dtype=np.float32).reshape(H, DK)

    wq_b = (Wq * SCALE).astype(BF)
    wk_b = Wk.astype(BF)
    wv_b = Wv.astype(BF)
    # relb packed per head pair: rows 0:64 = even head bias, 64:128 = odd head
    relb = np.ascontiguousarray(
        bias.reshape(NHP, 2, DK).transpose(1, 2, 0).reshape(128, NHP))
    onesb = np.ones((128, NJC * H), BF)
    shared = {"wq": wq_b, "wk": wk_b, "wv": wv_b, "relb": relb, "wo": Wo,
              "bo": bo[None, :], "onesb": onesb}

    in_maps = []
    for c in range(8):
        b, half = c // 2, c % 2
        xt = np.ascontiguousarray(x[b].T)              # [512, 2048]
        if half:
            xt = np.ascontiguousarray(np.roll(xt, -NI, axis=1))
        in_maps.append({"xt": xt.astype(BF), **shared})

    res = run_bass_kernel_spmd(nc, in_maps, core_ids=list(range(8)),
                               trace=_trace)
    out = np.empty((B, N, MODEL), np.float32)
    for c in range(8):
        b, half = c // 2, c % 2
        out[b, half * NI:(half + 1) * NI, :] = res.results[c]["y"]
    if _trace:
        return out, res
    return out


# revision 52
# speedup vs baseline: 1.0226x; 1.0226x over previous
"""Trainium2 Bass kernel for multi-head attention (b=4, n=2048, d=512, h=8, dk=dv=64).

Sharding: 8 cores = 4 batches x 2 query-halves. Each core computes K/V for its
full batch sequence (2048) and attention outputs for its 1024 query rows.
No collectives needed; host stacks the per-core [1024, 512] outputs.

Per-core dataflow:
  x^T [512, 2048] staged in SBUF as bf16; projections (bf16 MMs, f32 PSUM)
  emission-ordered against the HBM input stream.  Q/K projections are
  head-PAIR packed: one [128 = h_even dims | h_odd dims] PSUM tile per pair
  covers two heads per moving stream (halved MM columns, unreplicated wq/wk).
  Per-head S^T keeps full 128x128 stationaries via the zero-half trick:
  qt_h = [q+bias; 0] (or flipped) against a pair-shared kt, so the dead qt
  half contracts the other head's K rows away.  S^T/PV in f32r/bf16; exp on
  ScalarE from PSUM per [128,1024] chunk (the phase pacer).  The whole
  attention runs as ONE flat 128-chunk software pipeline: PV trails S^T by
  LOOK chunks across head boundaries, softmax-normalize runs inline, and all
  remaining projection + output-projection work is drip-fed between chunks in
  <=2-matmul slices from a dedicated PSUM pool so the exp stream never
  stalls.  Output projection accumulates per head-pair into SBUF (y_acc);
  the tail only adds pair 3 (h6's half early, h7's after its normalize) and
  streams y out in fp16.
"""
import numpy as np

B, N, MODEL = 4, 2048, 512
H, DK = 8, 64
SCALE = DK ** -0.5
NI = 1024           # query rows per core
NCH = MODEL // 128  # model-dim chunks
NJC = N // 128      # key/value chunks
NHP = H // 2        # head pairs
LOOK = 9            # PV chunk lookahead behind S^T

_COMPILED = None


def _build():
    import concourse.bass as bass
    from concourse import bacc
    import concourse.mybir as mybir
    import concourse.tile as tile

    F32 = mybir.dt.float32
    F32R = mybir.dt.float32r
    BF16 = mybir.dt.bfloat16
    F16 = mybir.dt.float16
    EXP = mybir.ActivationFunctionType.Exp
    AID = mybir.ActivationFunctionType.Identity

    nc = bacc.Bacc("TRN2", target_bir_lowering=False, debug=False, num_devices=8)
    xt_in = nc.dram_tensor("xt", [MODEL, N], BF16, kind="ExternalInput")
    wq_in = nc.dram_tensor("wq", [MODEL, MODEL], BF16, kind="ExternalInput")
    wk_in = nc.dram_tensor("wk", [MODEL, MODEL], BF16, kind="ExternalInput")
    wv_in = nc.dram_tensor("wv", [MODEL, MODEL], BF16, kind="ExternalInput")
    relb_in = nc.dram_tensor("relb", [128, NHP], F32, kind="ExternalInput")
    wo_in = nc.dram_tensor("wo", [MODEL, MODEL], F32R, kind="ExternalInput")
    bo_in = nc.dram_tensor("bo", [1, MODEL], F32, kind="ExternalInput")
    onesb_in = nc.dram_tensor("onesb", [128, NJC * H], BF16, kind="ExternalInput")
    y_out = nc.dram_tensor("y", [NI, MODEL], F16, kind="ExternalOutput")

    with tile.TileContext(nc) as tc:
        with (
            tc.tile_pool(name="w", bufs=1) as wp,
            tc.tile_pool(name="acts", bufs=1) as ap,
            tc.tile_pool(name="big", bufs=2, space="PSUM") as ps,
            tc.tile_pool(name="qk", bufs=2, space="PSUM") as qkp,
        ):
            # ---------- persistent tiles ----------
            wo = wp.tile([128, NCH, MODEL], F32R, tag="wo")
            bo = wp.tile([1, MODEL], F32, tag="bo")
            bo_b = wp.tile([128, MODEL], F32, tag="bo_b")
            vv_a = ap.tile([128, NJC // 2, H * 65], BF16, tag="vva")
            vv_b = ap.tile([128, NJC // 2, H * 65], BF16, tag="vvb")
            def vvt(jc):
                return (vv_a if jc < NJC // 2 else vv_b)[:, jc % (NJC // 2)]
            relb = ap.tile([128, NHP], F32, tag="relb")
            outt = ap.tile([128, NCH, NI], F32R, tag="outt")
            kt = ap.tile([128, NHP, NJC, 128], F32R, tag="kt")
            qt = ap.tile([128, H, NI], F32R, tag="qt")

            def r3(d):
                return d[:].rearrange("(c p) n -> p c n", p=128)

            dma_n = [0]
            def dma(out, in_):
                engs = (nc.sync, nc.gpsimd, nc.scalar)
                engs[dma_n[0] % 3].dma_start(out=out, in_=in_)
                dma_n[0] += 1
            def dma2(out, in_):
                # split in half over two queues for faster arrival
                dma(out[:, 0:2], in_[:, 0:2])
                dma(out[:, 2:4], in_[:, 2:4])

            with tc.tile_pool(name="proj", bufs=1) as pp:
                xt0 = pp.tile([128, NCH, 512], BF16, tag="xt0")
                xt1 = pp.tile([128, NCH, 512], BF16, tag="xt1")
                xt2 = pp.tile([128, NCH, 512], BF16, tag="xt2")
                xt3 = pp.tile([128, NCH, 512], BF16, tag="xt3")
                xts = [xt0, xt1, xt2, xt3]
                wq = pp.tile([128, NCH, MODEL], BF16, tag="wq")
                wk = pp.tile([128, NCH, MODEL], BF16, tag="wk")
                wv = pp.tile([128, NCH, MODEL], BF16, tag="wv")
                onesb_t = pp.tile([128, NJC * H], BF16, tag="onesb")

                # ---- DMA emission: one descriptor per tensor, priority
                # order round-robined over 3 queues ----
                xsrc = r3(xt_in)
                def dma_x(q):
                    dma2(xts[q][:], xsrc[:, :, q * 512:(q + 1) * 512])
                dma(relb[:], relb_in[:])
                dma(bo[:], bo_in[:])
                dma(onesb_t[:], onesb_in[:])
                dma2(wv[:], r3(wv_in))
                dma_x(0)
                dma(wq[:, :, 0:128], r3(wq_in)[:, :, 0:128])
                dma_x(1)
                dma(wk[:, :, 0:128], r3(wk_in)[:, :, 0:128])
                dma_x(2)
                dma_x(3)
                dma2(wq[:, :, 128:512], r3(wq_in)[:, :, 128:512])
                dma2(wk[:, :, 128:512], r3(wk_in)[:, :, 128:512])
                dma(wo[:], r3(wo_in))
                # HAM warm-up: accumulating matmuls on a zeroed scratch keep
                # the PE activity monitor busy while the input stream lands
                warm = pp.tile([128, 640], BF16, tag="warm")
                nc.vector.memset(warm[:], 0.0)
                w_ps = qkp.tile([128, MODEL], F32, tag="qk", name="w_ps")
                for w in range(12):
                    nc.tensor.matmul(w_ps[:], warm[:, 0:128], warm[:, 128:640],
                                     start=(w == 0), stop=(w == 11))
                # zero qt up front (the dead half of the zero-half trick must
                # be zero to mask the other head's K rows in the shared kt;
                # live halves are overwritten by the Q projection drains)
                for h in range(H):
                    nc.gpsimd.memset(qt[:, h, :].bitcast(F32), 0.0)
                nc.gpsimd.partition_broadcast(bo_b[:], bo[:])
                # ones columns of V_aug: contiguous DMA to scratch, strided copy
                for vh in range(2):
                    nc.vector.tensor_copy(
                        (vv_a if vh == 0 else vv_b)[:]
                        .rearrange("p j (h e) -> p (j h) e", e=65)[:, :, 64:65],
                        onesb_t[:, vh * NJC * H // 2:(vh + 1) * NJC * H // 2]
                        .rearrange("p (n o) -> p n o", o=1))

                def xtv(ch, start, size):
                    t = xts[start // 512]
                    off = start % 512
                    assert off + size <= 512
                    return t[:, ch, off:off + size]

                # ---- projection emitters, sliceable into 2-MM halves ----
                vps_st, qps_st, kps_st = {}, {}, {}

                def emit_v(jc, half):
                    if half == 0:
                        v_ps = qkp.tile([128, MODEL], F32, tag="qk",
                                        name="v_ps")
                        vps_st[jc] = v_ps
                    else:
                        v_ps = vps_st.pop(jc)
                    for ch in ((0, 1) if half == 0 else (2, 3)):
                        nc.tensor.matmul(v_ps[:],
                                         xtv(ch, jc * 128, 128),
                                         wv[:, ch],
                                         start=(ch == 0), stop=(ch == NCH - 1))
                    if half == 1:
                        nc.vector.tensor_copy(
                            vvt(jc).rearrange("p (h e) -> p h e", e=65)[:, :, 0:64],
                            v_ps[:].rearrange("p (h e) -> p h e", e=64))

                def emit_q(hp, ib, half, eng=None):
                    if half == 0:
                        q_ps = qkp.tile([128, MODEL], F32, tag="qk",
                                        name="q_ps")
                        qps_st[(hp, ib)] = q_ps
                    else:
                        q_ps = qps_st.pop((hp, ib))
                    for ch in ((0, 1) if half == 0 else (2, 3)):
                        nc.tensor.matmul(
                            q_ps[:, 0:512],
                            wq[:, ch, hp * 128:(hp + 1) * 128],
                            xtv(ch, ib * 512, 512),
                            start=(ch == 0), stop=(ch == NCH - 1))
                    if half == 1:
                        isl = slice(ib * 512, ib * 512 + 512)
                        if eng is None:
                            nc.vector.tensor_scalar_add(
                                qt[0:64, 2 * hp, isl], q_ps[0:64, 0:512],
                                relb[0:64, hp:hp + 1])
                            nc.vector.tensor_scalar_add(
                                qt[64:128, 2 * hp + 1, isl],
                                q_ps[64:128, 0:512],
                                relb[64:128, hp:hp + 1])
                        else:
                            eng.activation(qt[0:64, 2 * hp, isl],
                                           q_ps[0:64, 0:512], AID,
                                           bias=relb[0:64, hp:hp + 1])
                            eng.activation(qt[64:128, 2 * hp + 1, isl],
                                           q_ps[64:128, 0:512], AID,
                                           bias=relb[64:128, hp:hp + 1])

                def emit_k(hp, jb, sb, half, eng=None):
                    if half == 0:
                        k_ps = qkp.tile([128, MODEL], F32, tag="qk",
                                        name="k_ps")
                        kps_st[(hp, jb, sb)] = k_ps
                    else:
                        k_ps = kps_st.pop((hp, jb, sb))
                    off = jb * NI + sb * 512
                    for ch in ((0, 1) if half == 0 else (2, 3)):
                        nc.tensor.matmul(
                            k_ps[:, 0:512],
                            wk[:, ch, hp * 128:(hp + 1) * 128],
                            xtv(ch, off, 512),
                            start=(ch == 0), stop=(ch == NCH - 1))
                    if half == 1:
                        jcs = slice(jb * 8 + sb * 4, jb * 8 + sb * 4 + 4)
                        src = k_ps[:, 0:512].rearrange("p (j m) -> p j m", m=128)
                        if eng is None:
                            nc.vector.tensor_copy(kt[0:64, hp, jcs, :],
                                                  src[0:64])
                            nc.vector.tensor_copy(kt[64:128, hp, jcs, :],
                                                  src[64:128])
                        else:
                            eng.copy(kt[0:64, hp, jcs, :], src[0:64])
                            eng.copy(kt[64:128, hp, jcs, :], src[64:128])

                # ---- pre-ST: V0-3, K(0,jb0), Q0; K(0,jb1) goes to the
                # interleave (its x3 dependency lands after S^T starts) ----
                for jc in range(4):
                    emit_v(jc, 0)
                    emit_v(jc, 1)
                for sb in range(2):
                    emit_k(0, 0, sb, 0)
                    emit_k(0, 0, sb, 1, eng=nc.scalar)
                for ib in range(2):
                    emit_q(0, ib, 0)
                    emit_q(0, ib, 1, eng=nc.scalar)

                # ------ attention: flat pipeline over 128 S^T chunks with
                # drip-fed interleave ops ------
                with tc.tile_pool(name="pt", bufs=12) as ptp, \
                     tc.tile_pool(name="pv", bufs=1, space="PSUM") as pvp, \
                     tc.tile_pool(name="norm", bufs=2) as np_, \
                     tc.tile_pool(name="yac", bufs=1) as yac, \
                     tc.tile_pool(name="ysb", bufs=2) as yp_sb:
                    pts = {}
                    pvs = {}
                    y_acc = yac.tile([128, NI // 128, MODEL], F16, tag="yacc")

                    def emit_y(p, ib, hh):
                        # hh: None = full 128-row contraction (pairs 0-2);
                        # 0/1 = one head's 64 rows (pair 3 split around h7)
                        y_ps = qkp.tile([128, MODEL], F32, tag="qk",
                                        name="y_ps")
                        if hh is None:
                            nc.tensor.matmul(
                                y_ps[:], outt[:, p, ib * 128:(ib + 1) * 128],
                                wo[:, p], start=True, stop=True)
                        else:
                            b0 = hh * 64
                            nc.tensor.matmul(
                                y_ps[:],
                                outt[b0:b0 + 64, p, ib * 128:(ib + 1) * 128],
                                wo[b0:b0 + 64, p], start=True, stop=True)
                        if p == 0:
                            nc.vector.tensor_tensor(
                                out=y_acc[:, ib], in0=y_ps[:],
                                in1=bo_b[:], op=mybir.AluOpType.add)
                        elif p == 3:
                            y_sb = yp_sb.tile([128, MODEL], F16, tag="ysb")
                            nc.vector.tensor_tensor(
                                out=y_sb[:], in0=y_ps[:],
                                in1=y_acc[:, ib], op=mybir.AluOpType.add)
                            dma(y_out[ib * 128:(ib + 1) * 128, :], y_sb[:])
                        else:
                            nc.vector.tensor_tensor(
                                out=y_acc[:, ib], in0=y_ps[:],
                                in1=y_acc[:, ib], op=mybir.AluOpType.add)

                    def emit_st(g):
                        h, jc = g // NJC, g % NJC
                        st = ps.tile([128, NI], F32, tag="big")
                        for ih in range(2):
                            nc.tensor.matmul(
                                st[:, ih * 512:(ih + 1) * 512],
                                kt[:, h // 2, jc],
                                qt[:, h, ih * 512:(ih + 1) * 512],
                                start=True, stop=True)
                        pt = ptp.tile([128, NI], BF16, tag="pt")
                        pts[g] = pt
                        nc.scalar.activation(pt[:], st[:], EXP, scale=1.0)

                    def emit_pv(g):
                        h, jc = g // NJC, g % NJC
                        if jc == 0:
                            pv_t = pvp.tile([65, NI], F32, tag="pv",
                                            name="pv_t")
                            pvs[h] = pv_t
                        else:
                            pv_t = pvs[h]
                        pt = pts.pop(g)
                        for ih in range(2):
                            nc.tensor.matmul(
                                pv_t[:, ih * 512:(ih + 1) * 512],
                                vvt(jc)[:, h * 65:(h + 1) * 65],
                                pt[:, ih * 512:(ih + 1) * 512],
                                start=(jc == 0), stop=(jc == NJC - 1))
                        if jc == NJC - 1:
                            emit_norm(h)

                    def emit_norm(h):
                        hp, base = h // 2, (h % 2) * 64
                        pv_t = pvs.pop(h)
                        if h < 7:
                            # stage numerator to SBUF so the PSUM tile frees
                            # before the broadcast chain finishes
                            num = np_.tile([64, NI], F32, tag="num")
                            nc.vector.tensor_copy(num[:], pv_t[0:64, :])
                            num = num[:]
                        else:
                            num = pv_t[0:64, :]
                        den = np_.tile([1, NI], F32, tag="den")
                        nc.vector.tensor_copy(den[:], pv_t[64:65, :])
                        rrow = np_.tile([1, NI], F32, tag="rrow")
                        nc.vector.reciprocal_approx_fast(rrow[:], den[:])
                        rb = np_.tile([64, NI], F32, tag="rb")
                        nc.gpsimd.partition_broadcast(rb[:], rrow[:])
                        nc.vector.tensor_tensor(
                            out=outt[base:base + 64, hp, :],
                            in0=num, in1=rb[:],
                            op=mybir.AluOpType.mult)

                    # ---- interleave schedule: op lists keyed by g ----
                    ilv = {}
                    def at(g, fn):
                        ilv.setdefault(g, []).append(fn)
                    for sb in range(2):           # K(0,jb1) right after start
                        for half in range(2):
                            at(0, lambda sb=sb, half=half:
                               emit_k(0, 1, sb, half))
                    for jc in range(4, 16):       # V chunks 4..15, halved
                        at(2 * jc - 7, lambda jc=jc: emit_v(jc, 0))
                        at(2 * jc - 6, lambda jc=jc: emit_v(jc, 1))
                    def sched_qk(hp, g0):
                        g = g0
                        for ib in range(2):
                            for half in range(2):
                                at(g, lambda hp=hp, ib=ib, half=half:
                                   emit_q(hp, ib, half))
                                g += 1
                        for jb in range(2):
                            for sb in range(2):
                                for half in range(2):
                                    at(g, lambda hp=hp, jb=jb, sb=sb,
                                       half=half: emit_k(hp, jb, sb, half))
                                    g += 1
                    sched_qk(1, 25)
                    sched_qk(2, 49)
                    sched_qk(3, 81)
                    # y-partials trail pair p's normalize (g = 32p + 40)
                    for p in range(3):
                        for ib in range(8):
                            at(32 * p + 41 + ib,
                               lambda p=p, ib=ib: emit_y(p, ib, None))
                    pv_next = [0]
                    def drain_pv(upto):
                        while pv_next[0] <= min(upto, H * NJC - 1):
                            emit_pv(pv_next[0])
                            pv_next[0] += 1
                    for g in range(H * NJC):
                        emit_st(g)
                        for fn in ilv.get(g, ()):
                            fn()
                        # PV trails by LOOK; near the end, catch up so the
                        # drain overlaps the last exps
                        lag = LOOK if g < 120 else LOOK - (g - 119)
                        drain_pv(g - max(lag, 1))
                    drain_pv(H * NJC - 1)
                    # keep the PE clock warm through h7's normalize so the
                    # final output-projection matmuls run at full rate
                    w_ps2 = qkp.tile([128, MODEL], F32, tag="qk", name="w_ps2")
                    for w in range(14):
                        nc.tensor.matmul(w_ps2[:], warm[:, 0:128],
                                         warm[:, 128:640],
                                         start=(w == 0), stop=(w == 13))

                    # ---- tail: pair 3 + writeback ----
                    for ib in range(8):
                        emit_y(3, ib, None)

    nc.compile()
    return nc


def _get_compiled():
    global _COMPILED
    if _COMPILED is None:
        _COMPILED = _build()
    return _COMPILED


def kernel(x, Wq, Wk, Wv, Wo, bo, rel_content_bias, _trace=False):
    from concourse.bass_utils import run_bass_kernel_spmd
    import ml_dtypes

    nc = _get_compiled()
    BF = ml_dtypes.bfloat16

    x = np.asarray(x, dtype=np.float32)
    Wq = np.asarray(Wq, dtype=np.float32)
    Wk = np.asarray(Wk, dtype=np.float32)
    Wv = np.asarray(Wv, dtype=np.float32)
    Wo = np.asarray(Wo, dtype=np.float32)
    bo = np.asarray(bo, dtype=np.float32)
    bias = np.asarray(rel_content_bias, dtype=np.float32).reshape(H, DK)

    wq_b = (Wq * SCALE).astype(BF)
    wk_b = Wk.astype(BF)
    wv_b = Wv.astype(BF)
    # relb packed per head pair: rows 0:64 = even head bias, 64:128 = odd head
    relb = np.ascontiguousarray(
        bias.reshape(NHP, 2, DK).transpose(1, 2, 0).reshape(128, NHP))
    onesb = np.ones((128, NJC * H), BF)
    shared = {"wq": wq_b, "wk": wk_b, "wv": wv_b, "relb": relb, "wo": Wo,
              "bo": bo[None, :], "onesb": onesb}

    in_maps = []
    for c in range(8):
        b, half = c // 2, c % 2
        xt = np.ascontiguousarray(x[b].T)              # [512, 2048]
        if half:
            xt = np.ascontiguousarray(np.roll(xt, -NI, axis=1))
        in_maps.append({"xt": xt.astype(BF), **shared})

    res = run_bass_kernel_spmd(nc, in_maps, core_ids=list(range(8)),
                               trace=_trace)
    out = np.empty((B, N, MODEL), np.float32)
    for c in range(8):
        b, half = c // 2, c % 2
        out[b, half * NI:(half + 1) * NI, :] = res.results[c]["y"]
    if _trace:
        return out, res
    return out


# revision 54
# speedup vs baseline: 1.0275x; 1.0048x over previous
"""Trainium2 Bass kernel for multi-head attention (b=4, n=2048, d=512, h=8, dk=dv=64).

Sharding: 8 cores = 4 batches x 2 query-halves. Each core computes K/V for its
full batch sequence (2048) and attention outputs for its 1024 query rows.
No collectives needed; host stacks the per-core [1024, 512] outputs.

Per-core dataflow:
  x^T [512, 2048] staged in SBUF as bf16; projections (bf16 MMs, f32 PSUM)
  emission-ordered against the HBM input stream.  Q/K projections are
  head-PAIR packed: one [128 = h_even dims | h_odd dims] PSUM tile per pair
  covers two heads per moving stream (halved MM columns, unreplicated wq/wk).
  Per-head S^T keeps full 128x128 stationaries via the zero-half trick:
  qt_h = [q+bias; 0] (or flipped) against a pair-shared kt, so the dead qt
  half contracts the other head's K rows away.  S^T/PV in f32r/bf16; exp on
  ScalarE from PSUM per [128,1024] chunk (the phase pacer).  The whole
  attention runs as ONE flat 128-chunk software pipeline: PV trails S^T by
  LOOK chunks across head boundaries, softmax-normalize runs inline, and all
  remaining projection + output-projection work is drip-fed between chunks in
  <=2-matmul slices from a dedicated PSUM pool so the exp stream never
  stalls.  Output projection accumulates per head-pair into SBUF (y_acc);
  the tail only adds pair 3 (h6's half early, h7's after its normalize) and
  streams y out in fp16.
"""
import numpy as np

B, N, MODEL = 4, 2048, 512
H, DK = 8, 64
SCALE = DK ** -0.5
NI = 1024           # query rows per core
NCH = MODEL // 128  # model-dim chunks
NJC = N // 128      # key/value chunks
NHP = H // 2        # head pairs
LOOK = 9            # PV chunk lookahead behind S^T

_COMPILED = None


def _build():
    import concourse.bass as bass
    from concourse import bacc
    import concourse.mybir as mybir
    import concourse.tile as tile

    F32 = mybir.dt.float32
    F32R = mybir.dt.float32r
    BF16 = mybir.dt.bfloat16
    F16 = mybir.dt.float16
    EXP = mybir.ActivationFunctionType.Exp
    AID = mybir.ActivationFunctionType.Identity

    nc = bacc.Bacc("TRN2", target_bir_lowering=False, debug=False, num_devices=8)
    xt_in = nc.dram_tensor("xt", [MODEL, N], BF16, kind="ExternalInput")
    wq_in = nc.dram_tensor("wq", [MODEL, MODEL], BF16, kind="ExternalInput")
    wk_in = nc.dram_tensor("wk", [MODEL, MODEL], BF16, kind="ExternalInput")
    wv_in = nc.dram_tensor("wv", [MODEL, MODEL], BF16, kind="ExternalInput")
    relb_in = nc.dram_tensor("relb", [128, NHP], F32, kind="ExternalInput")
    wo_in = nc.dram_tensor("wo", [MODEL, MODEL], F32R, kind="ExternalInput")
    bo_in = nc.dram_tensor("bo", [1, MODEL], F32, kind="ExternalInput")
    onesb_in = nc.dram_tensor("onesb", [128, NJC * H], BF16, kind="ExternalInput")
    y_out = nc.dram_tensor("y", [NI, MODEL], F16, kind="ExternalOutput")

    with tile.TileContext(nc) as tc:
        with (
            tc.tile_pool(name="w", bufs=1) as wp,
            tc.tile_pool(name="acts", bufs=1) as ap,
            tc.tile_pool(name="big", bufs=2, space="PSUM") as ps,
            tc.tile_pool(name="qk", bufs=2, space="PSUM") as qkp,
        ):
            # ---------- persistent tiles ----------
            wo = wp.tile([128, NCH, MODEL], F32R, tag="wo")
            bo = wp.tile([1, MODEL], F32, tag="bo")
            bo_b = wp.tile([128, MODEL], F32, tag="bo_b")
            vv_a = ap.tile([128, NJC // 2, H * 65], BF16, tag="vva")
            vv_b = ap.tile([128, NJC // 2, H * 65], BF16, tag="vvb")
            def vvt(jc):
                return (vv_a if jc < NJC // 2 else vv_b)[:, jc % (NJC // 2)]
            relb = ap.tile([128, NHP], F32, tag="relb")
            outt = ap.tile([128, NCH, NI], F32R, tag="outt")
            kt = ap.tile([128, NHP, NJC, 128], F32R, tag="kt")
            qt = ap.tile([128, H, NI], F32R, tag="qt")

            def r3(d):
                return d[:].rearrange("(c p) n -> p c n", p=128)

            dma_n = [0]
            def dma(out, in_):
                engs = (nc.sync, nc.gpsimd, nc.scalar)
                engs[dma_n[0] % 3].dma_start(out=out, in_=in_)
                dma_n[0] += 1
            def dma2(out, in_):
                # split in half over two queues for faster arrival
                dma(out[:, 0:2], in_[:, 0:2])
                dma(out[:, 2:4], in_[:, 2:4])

            with tc.tile_pool(name="proj", bufs=1) as pp:
                xt0 = pp.tile([128, NCH, 512], BF16, tag="xt0")
                xt1 = pp.tile([128, NCH, 512], BF16, tag="xt1")
                xt2 = pp.tile([128, NCH, 512], BF16, tag="xt2")
                xt3 = pp.tile([128, NCH, 512], BF16, tag="xt3")
                xts = [xt0, xt1, xt2, xt3]
                wq = pp.tile([128, NCH, MODEL], BF16, tag="wq")
                wk = pp.tile([128, NCH, MODEL], BF16, tag="wk")
                wv = pp.tile([128, NCH, MODEL], BF16, tag="wv")
                onesb_t = pp.tile([128, NJC * H], BF16, tag="onesb")

                # ---- DMA emission: one descriptor per tensor, priority
                # order round-robined over 3 queues ----
                xsrc = r3(xt_in)
                def dma_x(q):
                    dma2(xts[q][:], xsrc[:, :, q * 512:(q + 1) * 512])
                dma(relb[:], relb_in[:])
                dma(bo[:], bo_in[:])
                dma(onesb_t[:], onesb_in[:])
                dma2(wv[:], r3(wv_in))
                dma_x(0)
                dma(wq[:, :, 0:128], r3(wq_in)[:, :, 0:128])
                dma_x(1)
                dma(wk[:, :, 0:128], r3(wk_in)[:, :, 0:128])
                dma_x(2)
                dma_x(3)
                dma2(wq[:, :, 128:512], r3(wq_in)[:, :, 128:512])
                dma2(wk[:, :, 128:512], r3(wk_in)[:, :, 128:512])
                dma(wo[:], r3(wo_in))
                # HAM warm-up: accumulating matmuls on a zeroed scratch keep
                # the PE activity monitor busy while the input stream lands
                warm = pp.tile([128, 640], BF16, tag="warm")
                nc.vector.memset(warm[:], 0.0)
                w_ps = qkp.tile([128, MODEL], F32, tag="qk", name="w_ps")
                for w in range(12):
                    nc.tensor.matmul(w_ps[:], warm[:, 0:128], warm[:, 128:640],
                                     start=(w == 0), stop=(w == 11))
                # zero qt up front (the dead half of the zero-half trick must
                # be zero to mask the other head's K rows in the shared kt;
                # live halves are overwritten by the Q projection drains)
                for h in range(H):
                    nc.gpsimd.memset(qt[:, h, :].bitcast(F32), 0.0)
                nc.gpsimd.partition_broadcast(bo_b[:], bo[:])
                # ones columns of V_aug: contiguous DMA to scratch, strided copy
                for vh in range(2):
                    nc.vector.tensor_copy(
                        (vv_a if vh == 0 else vv_b)[:]
                        .rearrange("p j (h e) -> p (j h) e", e=65)[:, :, 64:65],
                        onesb_t[:, vh * NJC * H // 2:(vh + 1) * NJC * H // 2]
                        .rearrange("p (n o) -> p n o", o=1))

                def xtv(ch, start, size):
                    t = xts[start // 512]
                    off = start % 512
                    assert off + size <= 512
                    return t[:, ch, off:off + size]

                # ---- projection emitters, sliceable into 2-MM halves ----
                vps_st, qps_st, kps_st = {}, {}, {}

                def emit_v(jc, half):
                    if half == 0:
                        v_ps = qkp.tile([128, MODEL], F32, tag="qk",
                                        name="v_ps")
                        vps_st[jc] = v_ps
                    else:
                        v_ps = vps_st.pop(jc)
                    for ch in ((0, 1) if half == 0 else (2, 3)):
                        nc.tensor.matmul(v_ps[:],
                                         xtv(ch, jc * 128, 128),
                                         wv[:, ch],
                                         start=(ch == 0), stop=(ch == NCH - 1))
                    if half == 1:
                        nc.vector.tensor_copy(
                            vvt(jc).rearrange("p (h e) -> p h e", e=65)[:, :, 0:64],
                            v_ps[:].rearrange("p (h e) -> p h e", e=64))

                def emit_q(hp, ib, half, eng=None):
                    if half == 0:
                        q_ps = qkp.tile([128, MODEL], F32, tag="qk",
                                        name="q_ps")
                        qps_st[(hp, ib)] = q_ps
                    else:
                        q_ps = qps_st.pop((hp, ib))
                    for ch in ((0, 1) if half == 0 else (2, 3)):
                        nc.tensor.matmul(
                            q_ps[:, 0:512],
                            wq[:, ch, hp * 128:(hp + 1) * 128],
                            xtv(ch, ib * 512, 512),
                            start=(ch == 0), stop=(ch == NCH - 1))
                    if half == 1:
                        isl = slice(ib * 512, ib * 512 + 512)
                        if eng is None:
                            nc.vector.tensor_scalar_add(
                                qt[0:64, 2 * hp, isl], q_ps[0:64, 0:512],
                                relb[0:64, hp:hp + 1])
                            nc.vector.tensor_scalar_add(
                                qt[64:128, 2 * hp + 1, isl],
                                q_ps[64:128, 0:512],
                                relb[64:128, hp:hp + 1])
                        else:
                            eng.activation(qt[0:64, 2 * hp, isl],
                                           q_ps[0:64, 0:512], AID,
                                           bias=relb[0:64, hp:hp + 1])
                            eng.activation(qt[64:128, 2 * hp + 1, isl],
                                           q_ps[64:128, 0:512], AID,
                                           bias=relb[64:128, hp:hp + 1])

                def emit_k(hp, jb, sb, half, eng=None):
                    if half == 0:
                        k_ps = qkp.tile([128, MODEL], F32, tag="qk",
                                        name="k_ps")
                        kps_st[(hp, jb, sb)] = k_ps
                    else:
                        k_ps = kps_st.pop((hp, jb, sb))
                    off = jb * NI + sb * 512
                    for ch in ((0, 1) if half == 0 else (2, 3)):
                        nc.tensor.matmul(
                            k_ps[:, 0:512],
                            wk[:, ch, hp * 128:(hp + 1) * 128],
                            xtv(ch, off, 512),
                            start=(ch == 0), stop=(ch == NCH - 1))
                    if half == 1:
                        jcs = slice(jb * 8 + sb * 4, jb * 8 + sb * 4 + 4)
                        src = k_ps[:, 0:512].rearrange("p (j m) -> p j m", m=128)
                        if eng is None:
                            nc.vector.tensor_copy(kt[0:64, hp, jcs, :],
                                                  src[0:64])
                            nc.vector.tensor_copy(kt[64:128, hp, jcs, :],
                                                  src[64:128])
                        else:
                            eng.copy(kt[0:64, hp, jcs, :], src[0:64])
                            eng.copy(kt[64:128, hp, jcs, :], src[64:128])

                # ---- pre-ST: V0-3, K(0,jb0), Q0; K(0,jb1) goes to the
                # interleave (its x3 dependency lands after S^T starts) ----
                for jc in range(4):
                    emit_v(jc, 0)
                    emit_v(jc, 1)
                for sb in range(2):
                    emit_k(0, 0, sb, 0)
                    emit_k(0, 0, sb, 1, eng=nc.scalar)
                for ib in range(2):
                    emit_q(0, ib, 0)
                    emit_q(0, ib, 1, eng=nc.scalar)

                # ------ attention: flat pipeline over 128 S^T chunks with
                # drip-fed interleave ops ------
                with tc.tile_pool(name="pt", bufs=12) as ptp, \
                     tc.tile_pool(name="pv", bufs=1, space="PSUM") as pvp, \
                     tc.tile_pool(name="norm", bufs=2) as np_, \
                     tc.tile_pool(name="yac", bufs=1) as yac, \
                     tc.tile_pool(name="ysb", bufs=2) as yp_sb:
                    pts = {}
                    pvs = {}
                    y_acc = yac.tile([128, NI // 128, MODEL], F16, tag="yacc")

                    def emit_y(p, ib, hh):
                        # hh: None = full 128-row contraction (pairs 0-2);
                        # 0/1 = one head's 64 rows (pair 3 split around h7)
                        y_ps = qkp.tile([128, MODEL], F32, tag="qk",
                                        name="y_ps")
                        if hh is None:
                            nc.tensor.matmul(
                                y_ps[:], outt[:, p, ib * 128:(ib + 1) * 128],
                                wo[:, p], start=True, stop=True)
                        else:
                            b0 = hh * 64
                            nc.tensor.matmul(
                                y_ps[:],
                                outt[b0:b0 + 64, p, ib * 128:(ib + 1) * 128],
                                wo[b0:b0 + 64, p], start=True, stop=True)
                        if p == 0:
                            nc.vector.tensor_tensor(
                                out=y_acc[:, ib], in0=y_ps[:],
                                in1=bo_b[:], op=mybir.AluOpType.add)
                        elif p == 3:
                            y_sb = yp_sb.tile([128, MODEL], F16, tag="ysb")
                            nc.vector.tensor_tensor(
                                out=y_sb[:], in0=y_ps[:],
                                in1=y_acc[:, ib], op=mybir.AluOpType.add)
                            dma(y_out[ib * 128:(ib + 1) * 128, :], y_sb[:])
                        else:
                            nc.vector.tensor_tensor(
                                out=y_acc[:, ib], in0=y_ps[:],
                                in1=y_acc[:, ib], op=mybir.AluOpType.add)

                    def emit_st(g):
                        h, jc = g // NJC, g % NJC
                        st = ps.tile([128, NI], F32, tag="big")
                        for ih in range(2):
                            nc.tensor.matmul(
                                st[:, ih * 512:(ih + 1) * 512],
                                kt[:, h // 2, jc],
                                qt[:, h, ih * 512:(ih + 1) * 512],
                                start=True, stop=True)
                        pt = ptp.tile([128, NI], BF16, tag="pt")
                        pts[g] = pt
                        nc.scalar.activation(pt[:], st[:], EXP, scale=1.0)

                    def emit_pv(g):
                        h, jc = g // NJC, g % NJC
                        if jc == 0:
                            pv_t = pvp.tile([65, NI], F32, tag="pv",
                                            name="pv_t")
                            pvs[h] = pv_t
                        else:
                            pv_t = pvs[h]
                        pt = pts.pop(g)
                        for ih in range(2):
                            nc.tensor.matmul(
                                pv_t[:, ih * 512:(ih + 1) * 512],
                                vvt(jc)[:, h * 65:(h + 1) * 65],
                                pt[:, ih * 512:(ih + 1) * 512],
                                start=(jc == 0), stop=(jc == NJC - 1))
                        if jc == NJC - 1:
                            emit_norm(h)

                    def emit_norm(h):
                        hp, base = h // 2, (h % 2) * 64
                        pv_t = pvs.pop(h)
                        if h < 7:
                            # stage numerator to SBUF so the PSUM tile frees
                            # before the broadcast chain finishes
                            num = np_.tile([64, NI], F32, tag="num")
                            nc.vector.tensor_copy(num[:], pv_t[0:64, :])
                            num = num[:]
                        else:
                            num = pv_t[0:64, :]
                        den = np_.tile([1, NI], F32, tag="den")
                        nc.vector.tensor_copy(den[:], pv_t[64:65, :])
                        rrow = np_.tile([1, NI], F32, tag="rrow")
                        nc.vector.reciprocal_approx_fast(rrow[:], den[:])
                        rb = np_.tile([64, NI], F32, tag="rb")
                        nc.gpsimd.partition_broadcast(rb[:], rrow[:])
                        nc.vector.tensor_tensor(
                            out=outt[base:base + 64, hp, :],
                            in0=num, in1=rb[:],
                            op=mybir.AluOpType.mult)

                    # ---- interleave schedule: op lists keyed by g ----
                    ilv = {}
                    def at(g, fn):
                        ilv.setdefault(g, []).append(fn)
                    for sb in range(2):           # K(0,jb1) right after start
                        for half in range(2):
                            at(0, lambda sb=sb, half=half:
                               emit_k(0, 1, sb, half))
                    for jc in range(4, 16):       # V chunks 4..15, halved
                        at(2 * jc - 7, lambda jc=jc: emit_v(jc, 0))
                        at(2 * jc - 6, lambda jc=jc: emit_v(jc, 1))
                    def sched_qk(hp, g0):
                        g = g0
                        for ib in range(2):
                            for half in range(2):
                                at(g, lambda hp=hp, ib=ib, half=half:
                                   emit_q(hp, ib, half))
                                g += 1
                        for jb in range(2):
                            for sb in range(2):
                                for half in range(2):
                                    at(g, lambda hp=hp, jb=jb, sb=sb,
                                       half=half: emit_k(hp, jb, sb, half))
                                    g += 1
                    sched_qk(1, 25)
                    sched_qk(2, 49)
                    sched_qk(3, 81)
                    # y-partials trail pair p's normalize (g = 32p + 40)
                    for p in range(3):
                        for ib in range(8):
                            at(32 * p + 41 + ib,
                               lambda p=p, ib=ib: emit_y(p, ib, None))
                    pv_next = [0]
                    def drain_pv(upto):
                        while pv_next[0] <= min(upto, H * NJC - 1):
                            emit_pv(pv_next[0])
                            pv_next[0] += 1
                    for g in range(H * NJC):
                        emit_st(g)
                        for fn in ilv.get(g, ()):
                            fn()
                        # PV trails by LOOK; near the end, catch up so the
                        # drain overlaps the last exps
                        lag = LOOK if g < 120 else LOOK - (g - 119)
                        drain_pv(g - max(lag, 1))
                    drain_pv(H * NJC - 1)
                    # keep the PE clock warm through h7's normalize so the
                    # final output-projection matmuls run at full rate
                    w_ps2 = qkp.tile([128, MODEL], F32, tag="qk", name="w_ps2")
                    for w in range(14):
                        nc.tensor.matmul(w_ps2[:], warm[:, 0:128],
                                         warm[:, 128:640],
                                         start=(w == 0), stop=(w == 13))

                    # ---- tail: pair 3 + writeback ----
                    for ib in range(8):
                        emit_y(3, ib, None)

    nc.compile()
    return nc


def _get_compiled():
    global _COMPILED
    if _COMPILED is None:
        _COMPILED = _build()
    return _COMPILED


def kernel(x, Wq, Wk, Wv, Wo, bo, rel_content_bias, _trace=False):
    from concourse.bass_utils import run_bass_kernel_spmd
    import ml_dtypes

    nc = _get_compiled()
    BF = ml_dtypes.bfloat16

    x = np.asarray(x, dtype=np.float32)
    Wq = np.asarray(Wq, dtype=np.float32)
    Wk = np.asarray(Wk, dtype=np.float32)
    Wv = np.asarray(Wv, dtype=np.float32)
    Wo = np.asarray(Wo, dtype=np.float32)
    bo = np.asarray(bo, dtype=np.float32)
    bias = np.asarray(rel_content_bias, dtype=np.float32).reshape(H, DK)

    wq_b = (Wq * SCALE).astype(BF)
    wk_b = Wk.astype(BF)
    wv_b = Wv.astype(BF)
    # relb packed per head pair: rows 0:64 = even head bias, 64:128 = odd head
    relb = np.ascontiguousarray(
        bias.reshape(NHP, 2, DK).transpose(1, 2, 0).reshape(128, NHP))
    onesb = np.ones((128, NJC * H), BF)
    shared = {"wq": wq_b, "wk": wk_b, "wv": wv_b, "relb": relb, "wo": Wo,
              "bo": bo[None, :], "onesb": onesb}

    in_maps = []
    for c in range(8):
        b, half = c // 2, c % 2
        xt = np.ascontiguousarray(x[b].T)              # [512, 2048]
        if half:
            xt = np.ascontiguousarray(np.roll(xt, -NI, axis=1))
        in_maps.append({"xt": xt.astype(BF), **shared})

    res = run_bass_kernel_spmd(nc, in_maps, core_ids=list(range(8)),
                               trace=_trace)
    out = np.empty((B, N, MODEL), np.float32)
    for c in range(8):
        b, half = c // 2, c % 2
        out[b, half * NI:(half + 1) * NI, :] = res.results[c]["y"]
    if _trace:
        return out, res
    return out
